# revision 30
# baseline (speedup 1.0000x reference)
"""Trainium2 Bass kernel for a dense transformer block (attention + MLP).

Strategy: data-parallel over batch across 8 NeuronCores (48 batches each).
Per core, batches are processed in groups of 4 (512 tokens) so every dense
matmul has a 512-wide moving operand. Activations live transposed in SBUF
([feature, token]) so DRAM-layout weights serve directly as the stationary
matmul operand. Matmul operands are fp16 (full PE rate, 10-bit mantissa);
accumulation is fp32 in PSUM.

Attention is computed k-major to avoid PE transposes entirely: scores are
built as S^T = K_h^T-stationary x Q_h-moving giving [t, s] tiles, exp is
taken UNnormalized into bfloat16 (fp32-range exponent, so no max-subtraction
is needed), the softmax denominators come from a ones-stationary matmul that
broadcasts column sums across all partitions, and normalization happens for
free in the PSUM->SBUF copy after the attnV matmul (scalar_tensor_tensor
multiply by the fast-reciprocal of the sums). The per-batch dependency chain
is scores -> exp -> attnV with everything else off the critical path.

Groups are software-pipelined: group i's attention is followed by group
i-1's MLP1 so the out-projection of group i never waits on the softmax
chain, and MLP2 of group i-1 closes the group.
"""

from contextlib import ExitStack

import numpy as np

B, S, E, H, D, F = 384, 128, 512, 4, 128, 2048
NCORES = 8
BL = B // NCORES  # 48 batches per core
GB = 4  # batches per group
NTOK = GB * S  # 512 tokens per group
KE = E // 128  # 4
KF = F // 128  # 16

MMDT_NP = np.float16  # matmul operand dtype (fp16: full PE rate, 10-bit mantissa)

_cache: dict = {}


# --------------------------------------------------------------------------
# Workaround: the walrus build in this container accepts at most ONE
# sync-wait command per instruction, while Tile emits several. Hoist every
# extra wait onto its own preceding same-engine InstNoOp (engine queues are
# FIFO, so this is semantically identical).
def _fix_multiwaits(nc):
    import concourse.mybir as mybir

    n = 0
    for fn in nc.m.functions:
        for bb in fn.blocks:
            out = []
            changed = False
            for inst in bb.instructions:
                si = inst.sync_info
                if si is not None and len(si.on_wait) > 1:
                    waits = list(si.on_wait)
                    for w in waits[:-1]:
                        n += 1
                        out.append(
                            mybir.InstNoOp(
                                name=f"I-mwfix-{n}",
                                engine=inst.engine,
                                bass_nofuse=True,
                                sync_info=mybir.SyncInfo(on_wait=[w], on_update=[]),
                            )
                        )
                    inst.sync_info = mybir.SyncInfo(
                        on_wait=[waits[-1]], on_update=list(si.on_update)
                    )
                    changed = True
                out.append(inst)
            if changed:
                bb.instructions = out
    return n


def _build(ng, variant="full", repeat=1, cfg=None, mwfix=True):
    """Build the per-core Bass program processing ng groups of 4 batches."""
    import concourse.bass as bass
    import concourse.mybir as mybir
    import concourse.tile as tile

    cfg = {
        **dict(
            big=5, sz=2, atp=1,
            xtp=2, qkp=2, vp=2, pp=3, rzp=2, atsb=2, tmpp=3, xmp=2, hp=2, yp=3,
            nk8=2,  # leading k-tiles of MLP1 done in fp8 DoubleRow (0|2|4)
            nf8=4,  # leading f-tiles of MLP2 done in fp8 DoubleRow (even, 0..16)
        ),
        **(cfg or {}),
    }
    nk8 = cfg["nk8"]
    nf8 = cfg["nf8"]
    gb = GB
    ntok = NTOK
    f32 = mybir.dt.float32
    f16 = mybir.dt.float16
    b16 = mybir.dt.bfloat16
    AF = mybir.ActivationFunctionType
    ALU = mybir.AluOpType
    ts = bass.ts

    ntok_total = BL * S

    nc = bass.Bass("TRN2", target_bir_lowering=False, debug=False)

    xt = nc.dram_tensor("xt", [E, ntok_total], f16, kind="ExternalInput")
    wq_d = nc.dram_tensor("wq", [E, E], f16, kind="ExternalInput")
    wk_d = nc.dram_tensor("wk", [E, E], f16, kind="ExternalInput")
    wv_d = nc.dram_tensor("wv", [E, E], f16, kind="ExternalInput")
    wo_d = nc.dram_tensor("wo", [E, E], f16, kind="ExternalInput")
    w1_d = nc.dram_tensor("w1", [E, F], f16, kind="ExternalInput")
    w1q_d = nc.dram_tensor("w1q", [128, KE, F], mybir.dt.float8e4, kind="ExternalInput")
    w2_d = nc.dram_tensor("w2", [F, E], f16, kind="ExternalInput")
    w2q_d = nc.dram_tensor("w2q", [128, KF, E], mybir.dt.float8e4, kind="ExternalInput")
    bias_d = nc.dram_tensor("bias", [128, 32], f32, kind="ExternalInput")
    yt = nc.dram_tensor("yt", [E, ntok_total], f16, kind="ExternalOutput")

    with tile.TileContext(nc) as tc, ExitStack() as ctx:
        singles = ctx.enter_context(tc.tile_pool(name="singles", bufs=1))

        xtp = ctx.enter_context(tc.tile_pool(name="xtp", bufs=cfg["xtp"]))

        def load_weight(name, dram, n_k, width, engine):
            t = singles.tile([128, n_k, width], f16, tag=f"w_{name}", name=f"w_{name}")
            engine.dma_start(
                out=t, in_=dram[:, :].rearrange("(k p) w -> p k w", p=128)
            )
            return [t[:, k, :] for k in range(n_k)]

        # group 0's x load is emitted by the pipeline BEFORE these weight
        # loads land on the rings, so first matmuls start early.
        xt_tiles = {}

        def emit_load(i):
            g = i % ng
            c0 = g * ntok
            xt_t = xtp.tile([128, KE, ntok], f16, tag="xt", name="xt_t")
            nc.sync.dma_start(
                out=xt_t,
                in_=xt[:, c0 : c0 + ntok].rearrange("(k p) t -> p k t", p=128),
            )
            xt_tiles[i] = [xt_t[:, k, :] for k in range(KE)]

        emit_load(0)

        # q/k weights ride the sync ring right behind group 0's x; the rest
        # ride the (otherwise idle at prologue) vector/scalar rings.
        bias_sb = singles.tile([128, 32], f32, tag="b_all", name="b_all")
        nc.sync.dma_start(out=bias_sb, in_=bias_d[:, :])

        wq_sb = load_weight("wq", wq_d, KE, E, nc.sync)
        wk_sb = load_weight("wk", wk_d, KE, E, nc.sync)
        wv_sb = load_weight("wv", wv_d, KE, E, nc.scalar)
        wo_sb = load_weight("wo", wo_d, KE, E, nc.scalar)
        w1_sb = load_weight("w1", w1_d, KE, F, nc.scalar)
        w2_sb = load_weight("w2", w2_d, KF, E, nc.scalar)
        w1q_sb = singles.tile([128, KE, F], mybir.dt.float8e4, tag="w_w1q", name="w_w1q")
        if nk8:
            nc.scalar.dma_start(out=w1q_sb, in_=w1q_d[:, :, :])
        w2q_sb = singles.tile([128, KF, E], mybir.dt.float8e4, tag="w_w2q", name="w_w2q")
        if nf8:
            nc.scalar.dma_start(out=w2q_sb, in_=w2q_d[:, :, :])

        bq_sb = bias_sb[:, 0:KE]
        bk_sb = bias_sb[:, KE : 2 * KE]
        bo_sb = bias_sb[:, 2 * KE : 3 * KE]
        b1_sb = bias_sb[:, 12 : 12 + KF]
        b2_sb = bias_sb[:, 28 : 28 + KE]

        ones = singles.tile([128, 128], b16, tag="ones")
        nc.gpsimd.memset(ones, 1.0)

        qkp = ctx.enter_context(tc.tile_pool(name="qkp", bufs=cfg["qkp"]))
        vp = ctx.enter_context(tc.tile_pool(name="vp", bufs=cfg["vp"]))
        pp = ctx.enter_context(tc.tile_pool(name="pp", bufs=cfg["pp"]))
        rzp = ctx.enter_context(tc.tile_pool(name="rzp", bufs=cfg["rzp"]))
        atp = ctx.enter_context(tc.tile_pool(name="atp", bufs=cfg["atsb"]))
        tmpp = ctx.enter_context(tc.tile_pool(name="tmpp", bufs=cfg["tmpp"]))
        xmp = ctx.enter_context(tc.tile_pool(name="xmp", bufs=cfg["xmp"]))
        hp = ctx.enter_context(tc.tile_pool(name="hp", bufs=cfg["hp"]))
        yp = ctx.enter_context(tc.tile_pool(name="yp", bufs=cfg["yp"]))

        ps_big = ctx.enter_context(tc.tile_pool(name="ps_big", bufs=cfg["big"], space="PSUM"))
        ps_sz = ctx.enter_context(tc.tile_pool(name="ps_sz", bufs=cfg["sz"], space="PSUM"))
        ps_atp = ctx.enter_context(tc.tile_pool(name="ps_atp", bufs=cfg["atp"], space="PSUM"))

        n_iters = ng * repeat
        st = {}  # per-iteration state

        def emit_qk(i):
            xt_sb = xt_tiles[i]
            s = st[i] = {}
            q_sb, k_sb = [], []
            for which, w_sb, b_sb, dst in (
                ("q", wq_sb, bq_sb, q_sb),
                ("k", wk_sb, bk_sb, k_sb),
            ):
                for h in range(H):
                    ps = ps_big.tile([128, ntok], f32, tag="big", name="qk_ps")
                    for k in range(KE):
                        nc.tensor.matmul(
                            ps,
                            w_sb[k][:, ts(h, 128)],
                            xt_sb[k],
                            start=(k == 0),
                            stop=(k == KE - 1),
                        )
                    t = qkp.tile([128, ntok], f16, tag=f"{which}{h}", name=f"{which}{h}")
                    nc.vector.tensor_scalar_add(t, ps, b_sb[:, h : h + 1])
                    dst.append(t)
            s["q"], s["k"] = q_sb, k_sb

        def emit_v(i):
            s = st[i]
            xt_sb = xt_tiles[i]
            v_sb = []
            for bi in range(gb):
                ps = ps_big.tile([128, E], f32, tag="big", name="v_ps")
                for k in range(KE):
                    nc.tensor.matmul(
                        ps,
                        xt_sb[k][:, ts(bi, 128)],
                        wv_sb[k],
                        start=(k == 0),
                        stop=(k == KE - 1),
                    )
                t = vp.tile([128, E], b16, tag=f"v{bi}", name=f"v{bi}")
                nc.scalar.activation(out=t, in_=ps, func=AF.Copy)
                v_sb.append(t)
            s["v"] = v_sb
            at_t = atp.tile([128, H, ntok], f16, tag="at", name="at_t")
            s["at_t"] = at_t
            s["at"] = [at_t[:, h, :] for h in range(H)]

        def emit_scores(i, bi):
            # S^T[t, (h,s)] for batch bi, then unnormalized exp in bfloat16
            s = st[i]
            s_ps = ps_sz.tile([128, H * 128], f32, tag="sz", name="s_ps")
            for h in range(H):
                nc.tensor.matmul(
                    s_ps[:, ts(h, 128)],
                    s["k"][h][:, ts(bi, 128)],
                    s["q"][h][:, ts(bi, 128)],
                )
            p_sb = pp.tile([128, H * 128], b16, tag="p", name="p_sb")
            nc.scalar.activation(out=p_sb, in_=s_ps, func=AF.Exp)
            s[f"p{bi}"] = p_sb

        def emit_attn_out(i, bi):
            s = st[i]
            p_sb = s.pop(f"p{bi}")
            # column sums of exp, broadcast to all partitions by the
            # ones-stationary matmul
            z_ps = ps_sz.tile([128, H * 128], f32, tag="sz", name="z_ps")
            nc.tensor.matmul(z_ps, ones, p_sb)
            rz_sb = rzp.tile([128, H * 128], f32, tag="rz", name="rz_sb")
            nc.vector.reciprocal(rz_sb, z_ps)
            at_ps = ps_atp.tile([128, H, 128], f32, tag="atp", name="at_ps")
            for h in range(H):
                nc.tensor.matmul(
                    at_ps[:, h, :], s["v"][bi][:, ts(h, 128)], p_sb[:, ts(h, 128)]
                )
            # normalize while copying out of PSUM: at = at_ps * (1/z)
            nc.vector.tensor_mul(
                s["at_t"][:, :, ts(bi, 128)],
                at_ps,
                rz_sb.rearrange("p (h s) -> p h s", h=H),
            )

        def emit_outproj(i):
            s = st[i]
            xt_sb = xt_tiles.pop(i)
            xm_sb = []
            for m in range(KE):
                ps = ps_big.tile([128, ntok], f32, tag="big", name="o_ps")
                for k in range(KE):
                    nc.tensor.matmul(
                        ps,
                        wo_sb[k][:, ts(m, 128)],
                        s["at"][k],
                        start=(k == 0),
                        stop=(k == KE - 1),
                    )
                tmp = tmpp.tile([128, ntok], f16, tag="tmp", name="tmp")
                nc.scalar.activation(
                    out=tmp, in_=ps, func=AF.Identity, bias=bo_sb[:, m : m + 1]
                )
                xm = xmp.tile([128, ntok], f16, tag=f"xm{m}", name=f"xm{m}")
                nc.gpsimd.tensor_add(xm, tmp, xt_sb[m])
                xm_sb.append(xm)
            s["xm"] = xm_sb
            xm8 = []
            for j in range(nk8 // 2):
                t8 = xmp.tile([128, 2, ntok], mybir.dt.float8e4, tag=f"xm8_{j}", name=f"xm8_{j}")
                for r in range(2):
                    nc.gpsimd.tensor_copy(t8[:, r, :], xm_sb[2 * j + r])
                xm8.append(t8)
            s["xm8"] = xm8

        def emit_mlp1_chunk(i, fs):
            s = st[i]
            h_sb = s.setdefault("h", [])
            for f in fs:
                ps = ps_big.tile([128, ntok], f32, tag="big", name="h_ps")
                for j in range(nk8 // 2):
                    nc.tensor.matmul(
                        ps,
                        w1q_sb[:, 2 * j : 2 * j + 2, ts(f, 128)],
                        s["xm8"][j],
                        start=(j == 0),
                        stop=(nk8 == KE and j == nk8 // 2 - 1),
                        perf_mode=mybir.MatmulPerfMode.DoubleRow,
                    )
                for k in range(nk8, KE):
                    nc.tensor.matmul(
                        ps,
                        w1_sb[k][:, ts(f, 128)],
                        s["xm"][k],
                        start=(k == 0),
                        stop=(k == KE - 1),
                    )
                if f < nf8:
                    # fp8 pair tile feeding MLP2's DoubleRow chain directly
                    j = f // 2
                    if f % 2 == 0:
                        t8 = hp.tile(
                            [128, 2, ntok], mybir.dt.float8e4,
                            tag=f"h8_{j}", name=f"h8_{j}",
                        )
                        s.setdefault("h8", []).append(t8)
                    out_ap = s["h8"][j][:, f % 2, :]
                    t = None
                else:
                    t = hp.tile([128, ntok], f16, tag=f"h{f}", name=f"h{f}")
                    out_ap = t
                if f % 2 == 0:
                    nc.scalar.activation(
                        out=out_ap, in_=ps, func=AF.Relu, bias=b1_sb[:, f : f + 1]
                    )
                else:
                    nc.vector.tensor_scalar(
                        out_ap, ps, b1_sb[:, f : f + 1], 0.0,
                        op0=ALU.add, op1=ALU.max,
                    )
                h_sb.append(t)

        def emit_mlp2_store(i):
            s = st[i]
            g = i % ng
            c0 = g * ntok
            h_sb = s["h"]
            yf = yp.tile([128, KE, ntok], f16, tag="yf", name="yf")
            for m in range(KE):
                ps = ps_big.tile([128, ntok], f32, tag="big", name="acc_ps")
                for j in range(nf8 // 2):
                    nc.tensor.matmul(
                        ps,
                        w2q_sb[:, 2 * j : 2 * j + 2, ts(m, 128)],
                        s["h8"][j],
                        start=(j == 0),
                        stop=(nf8 == KF and j == nf8 // 2 - 1),
                        perf_mode=mybir.MatmulPerfMode.DoubleRow,
                    )
                for f in range(nf8, KF):
                    nc.tensor.matmul(
                        ps,
                        w2_sb[f][:, ts(m, 128)],
                        h_sb[f],
                        start=(f == 0),
                        stop=(f == KF - 1),
                    )
                tmpf = tmpp.tile([128, ntok], f32, tag="tmpf", name="tmpf")
                nc.scalar.activation(
                    out=tmpf, in_=ps, func=AF.Identity, scale=1.0 / 256.0,
                    bias=b2_sb[:, m : m + 1],
                )
                nc.gpsimd.tensor_add(yf[:, m, :], tmpf, s["xm"][m])
            nc.scalar.dma_start(
                out=yt[:, c0 : c0 + ntok].rearrange("(k p) t -> p k t", p=128),
                in_=yf,
            )
            del st[i]

        for i in range(n_iters):
            if i + 1 < n_iters:
                emit_load(i + 1)
            emit_qk(i)
            emit_v(i)
            for bi in range(gb):
                emit_scores(i, bi)
            for bi in range(gb):
                emit_attn_out(i, bi)
            if i >= 1:
                emit_mlp1_chunk(i - 1, range(KF))
            emit_outproj(i)
            if i >= 1:
                emit_mlp2_store(i - 1)
        emit_mlp1_chunk(n_iters - 1, range(KF))
        emit_mlp2_store(n_iters - 1)

    if mwfix:
        _fix_multiwaits(nc)
    return nc


def _get_program(ng, variant="full", repeat=1, cfg=None):
    key = ("nc", ng, variant, repeat, tuple(sorted((cfg or {}).items())))
    if key not in _cache:
        _cache[key] = _build(ng, variant, repeat, cfg)
    return _cache[key]


def make_in_maps(x, wq, bq, wk, bk, wv, bv, wo, bo, w1, b1, w2, b2):
    x = np.asarray(x, np.float32)
    to_bf = lambda a: np.ascontiguousarray(np.asarray(a, np.float32).astype(MMDT_NP))

    # host-side prep: shard + transpose + cast
    ntok_total = BL * S
    x_sh = x.reshape(NCORES, ntok_total, E)
    xts = [np.ascontiguousarray(x_sh[c].T).astype(MMDT_NP) for c in range(NCORES)]

    # MLP scale convention: w1 (and b1) carry x16 so the fp8 copy of w1 sits
    # in e4m3's normal range; h tiles then hold 16*h and w2 carries /16.
    # Both scalings are exact powers of two in fp16.
    import ml_dtypes

    w1_s = np.asarray(w1, np.float32) * 16.0
    w2_s = np.asarray(w2, np.float32) * 16.0
    wq_b, wk_b, wv_b, wo_b, w1_b, w2_b = map(
        to_bf, (wq, wk, wv, wo, w1_s, w2_s)
    )
    w1q = np.ascontiguousarray(
        w1_s.reshape(KE, 128, F).transpose(1, 0, 2)
    ).astype(ml_dtypes.float8_e4m3)
    w2q = np.ascontiguousarray(
        w2_s.reshape(KF, 128, E).transpose(1, 0, 2)
    ).astype(ml_dtypes.float8_e4m3)

    resh = lambda b, nk: np.asarray(b, np.float32).reshape(nk, 128).T
    # bv is folded into the output-projection bias: P rows sum to 1, so
    # attn@wo + bo == (P@v_nobias)@wo + (bv@wo + bo).
    bo_eff = (
        np.asarray(bv, np.float64) @ np.asarray(wo, np.float64)
        + np.asarray(bo, np.float64)
    ).astype(np.float32)
    bias_pack = np.zeros((128, 32), np.float32)
    bias_pack[:, 0:KE] = resh(bq, KE)
    bias_pack[:, KE : 2 * KE] = resh(bk, KE)
    bias_pack[:, 2 * KE : 3 * KE] = resh(bo_eff, KE)
    bias_pack[:, 12 : 12 + KF] = resh(np.asarray(b1, np.float32) * 16.0, KF)
    bias_pack[:, 28 : 28 + KE] = resh(b2, KE)

    in_maps = []
    for c in range(NCORES):
        in_maps.append(
            {
                "xt": xts[c],
                "wq": wq_b,
                "wk": wk_b,
                "wv": wv_b,
                "wo": wo_b,
                "w1": w1_b,
                "w1q": w1q,
                "w2": w2_b,
                "w2q": w2q,
                "bias": bias_pack,
            }
        )
    return in_maps


def kernel(
    x, wq, bq, wk, bk, wv, bv, wo, bo, w1, b1, w2, b2, _ng=BL // GB
):
    import os

    from concourse.bass_utils import run_bass_kernel_spmd

    # The NTFF trace hook module does not exist in this container; make sure
    # run_bass_kernel_spmd never takes the trace branch even if BASS_TRACE
    # is set in the environment.
    os.environ["BASS_NEVER_TRACE"] = "1"

    in_maps = make_in_maps(x, wq, bq, wk, bk, wv, bv, wo, bo, w1, b1, w2, b2)
    ntok_total = BL * S
    nc = _get_program(_ng)

    res = run_bass_kernel_spmd(nc, in_maps, core_ids=list(range(NCORES)))
    _cache["last_result"] = res

    out = np.empty((NCORES, ntok_total, E), np.float32)
    for c in range(NCORES):
        out[c] = res.results[c]["yt"].T.astype(np.float32)
    return out.reshape(B, S, E)


# revision 35
# speedup vs baseline: 1.0078x; 1.0078x over previous
"""Trainium2 Bass kernel for a dense transformer block (attention + MLP).

Strategy: data-parallel over batch across 8 NeuronCores (48 batches each).
Per core, batches are processed in groups of 4 (512 tokens) so every dense
matmul has a 512-wide moving operand. Activations live transposed in SBUF
([feature, token]) so DRAM-layout weights serve directly as the stationary
matmul operand. Matmul operands are fp16 (full PE rate, 10-bit mantissa);
accumulation is fp32 in PSUM.

Attention is computed k-major to avoid PE transposes entirely: scores are
built as S^T = K_h^T-stationary x Q_h-moving giving [t, s] tiles, exp is
taken UNnormalized into bfloat16 (fp32-range exponent, so no max-subtraction
is needed), the softmax denominators come from a ones-stationary matmul that
broadcasts column sums across all partitions, and normalization happens for
free in the PSUM->SBUF copy after the attnV matmul (scalar_tensor_tensor
multiply by the fast-reciprocal of the sums). The per-batch dependency chain
is scores -> exp -> attnV with everything else off the critical path.

Groups are software-pipelined: group i's attention is followed by group
i-1's MLP1 so the out-projection of group i never waits on the softmax
chain, and MLP2 of group i-1 closes the group.
"""

from contextlib import ExitStack

import numpy as np

B, S, E, H, D, F = 384, 128, 512, 4, 128, 2048
NCORES = 8
BL = B // NCORES  # 48 batches per core
GB = 4  # batches per group
NTOK = GB * S  # 512 tokens per group
KE = E // 128  # 4
KF = F // 128  # 16

MMDT_NP = np.float16  # matmul operand dtype (fp16: full PE rate, 10-bit mantissa)

_cache: dict = {}


# --------------------------------------------------------------------------
# Workaround: the walrus build in this container accepts at most ONE
# sync-wait command per instruction, while Tile emits several. Hoist every
# extra wait onto its own preceding same-engine InstNoOp (engine queues are
# FIFO, so this is semantically identical).
def _fix_multiwaits(nc):
    import concourse.mybir as mybir

    n = 0
    for fn in nc.m.functions:
        for bb in fn.blocks:
            out = []
            changed = False
            for inst in bb.instructions:
                si = inst.sync_info
                if si is not None and len(si.on_wait) > 1:
                    waits = list(si.on_wait)
                    for w in waits[:-1]:
                        n += 1
                        out.append(
                            mybir.InstNoOp(
                                name=f"I-mwfix-{n}",
                                engine=inst.engine,
                                bass_nofuse=True,
                                sync_info=mybir.SyncInfo(on_wait=[w], on_update=[]),
                            )
                        )
                    inst.sync_info = mybir.SyncInfo(
                        on_wait=[waits[-1]], on_update=list(si.on_update)
                    )
                    changed = True
                out.append(inst)
            if changed:
                bb.instructions = out
    return n


def _build(ng, variant="full", repeat=1, cfg=None, mwfix=True):
    """Build the per-core Bass program processing ng groups of 4 batches."""
    import concourse.bass as bass
    import concourse.mybir as mybir
    import concourse.tile as tile

    cfg = {
        **dict(
            big=5, sz=2, atp=1,
            xtp=2, qkp=2, vp=2, pp=3, rzp=2, atsb=2, tmpp=3, xmp=2, hp=2, yp=3,
            nk8=2,  # leading k-tiles of MLP1 done in fp8 DoubleRow (0|2|4)
            nf8=4,  # leading f-tiles of MLP2 done in fp8 DoubleRow (even, 0..16)
        ),
        **(cfg or {}),
    }
    nk8 = cfg["nk8"]
    nf8 = cfg["nf8"]
    gb = GB
    ntok = NTOK
    f32 = mybir.dt.float32
    f16 = mybir.dt.float16
    b16 = mybir.dt.bfloat16
    AF = mybir.ActivationFunctionType
    ALU = mybir.AluOpType
    ts = bass.ts

    ntok_total = BL * S

    nc = bass.Bass("TRN2", target_bir_lowering=False, debug=False)

    xt = nc.dram_tensor("xt", [E, ntok_total], f16, kind="ExternalInput")
    wq_d = nc.dram_tensor("wq", [E, E], f16, kind="ExternalInput")
    wk_d = nc.dram_tensor("wk", [E, E], f16, kind="ExternalInput")
    wv_d = nc.dram_tensor("wv", [E, E], f16, kind="ExternalInput")
    wo_d = nc.dram_tensor("wo", [E, E], f16, kind="ExternalInput")
    w1_d = nc.dram_tensor("w1", [E, F], f16, kind="ExternalInput")
    w1q_d = nc.dram_tensor("w1q", [128, KE, F], mybir.dt.float8e4, kind="ExternalInput")
    w2_d = nc.dram_tensor("w2", [F, E], f16, kind="ExternalInput")
    w2q_d = nc.dram_tensor("w2q", [128, KF, E], mybir.dt.float8e4, kind="ExternalInput")
    bias_d = nc.dram_tensor("bias", [128, 32], f32, kind="ExternalInput")
    yt = nc.dram_tensor("yt", [E, ntok_total], f16, kind="ExternalOutput")

    with tile.TileContext(nc) as tc, ExitStack() as ctx:
        singles = ctx.enter_context(tc.tile_pool(name="singles", bufs=1))

        xtp = ctx.enter_context(tc.tile_pool(name="xtp", bufs=cfg["xtp"]))

        def load_weight(name, dram, n_k, width, engine):
            t = singles.tile([128, n_k, width], f16, tag=f"w_{name}", name=f"w_{name}")
            engine.dma_start(
                out=t, in_=dram[:, :].rearrange("(k p) w -> p k w", p=128)
            )
            return [t[:, k, :] for k in range(n_k)]

        # group 0's x load is emitted by the pipeline BEFORE these weight
        # loads land on the rings, so first matmuls start early.
        xt_tiles = {}

        def emit_load(i):
            g = i % ng
            c0 = g * ntok
            xt_t = xtp.tile([128, KE, ntok], f16, tag="xt", name="xt_t")
            nc.sync.dma_start(
                out=xt_t,
                in_=xt[:, c0 : c0 + ntok].rearrange("(k p) t -> p k t", p=128),
            )
            xt_tiles[i] = [xt_t[:, k, :] for k in range(KE)]

        emit_load(0)

        # q/k weights ride the sync ring right behind group 0's x; the rest
        # ride the (otherwise idle at prologue) vector/scalar rings.
        bias_sb = singles.tile([128, 32], f32, tag="b_all", name="b_all")
        nc.sync.dma_start(out=bias_sb, in_=bias_d[:, :])

        wq_sb = load_weight("wq", wq_d, KE, E, nc.sync)
        wk_sb = load_weight("wk", wk_d, KE, E, nc.sync)
        wv_sb = load_weight("wv", wv_d, KE, E, nc.scalar)
        wo_sb = load_weight("wo", wo_d, KE, E, nc.scalar)
        w1_sb = load_weight("w1", w1_d, KE, F, nc.scalar)
        w2_sb = load_weight("w2", w2_d, KF, E, nc.scalar)
        w1q_sb = singles.tile([128, KE, F], mybir.dt.float8e4, tag="w_w1q", name="w_w1q")
        if nk8:
            nc.scalar.dma_start(out=w1q_sb, in_=w1q_d[:, :, :])
        w2q_sb = singles.tile([128, KF, E], mybir.dt.float8e4, tag="w_w2q", name="w_w2q")
        if nf8:
            nc.scalar.dma_start(out=w2q_sb, in_=w2q_d[:, :, :])

        bq_sb = bias_sb[:, 0:KE]
        bk_sb = bias_sb[:, KE : 2 * KE]
        bo_sb = bias_sb[:, 2 * KE : 3 * KE]
        b1_sb = bias_sb[:, 12 : 12 + KF]
        b2_sb = bias_sb[:, 28 : 28 + KE]

        ones = singles.tile([128, 128], b16, tag="ones")
        nc.gpsimd.memset(ones, 1.0)

        qkp = ctx.enter_context(tc.tile_pool(name="qkp", bufs=cfg["qkp"]))
        vp = ctx.enter_context(tc.tile_pool(name="vp", bufs=cfg["vp"]))
        pp = ctx.enter_context(tc.tile_pool(name="pp", bufs=cfg["pp"]))
        rzp = ctx.enter_context(tc.tile_pool(name="rzp", bufs=cfg["rzp"]))
        atp = ctx.enter_context(tc.tile_pool(name="atp", bufs=cfg["atsb"]))
        tmpp = ctx.enter_context(tc.tile_pool(name="tmpp", bufs=cfg["tmpp"]))
        xmp = ctx.enter_context(tc.tile_pool(name="xmp", bufs=cfg["xmp"]))
        hp = ctx.enter_context(tc.tile_pool(name="hp", bufs=cfg["hp"]))
        yp = ctx.enter_context(tc.tile_pool(name="yp", bufs=cfg["yp"]))

        ps_big = ctx.enter_context(tc.tile_pool(name="ps_big", bufs=cfg["big"], space="PSUM"))
        ps_sz = ctx.enter_context(tc.tile_pool(name="ps_sz", bufs=cfg["sz"], space="PSUM"))
        ps_atp = ctx.enter_context(tc.tile_pool(name="ps_atp", bufs=cfg["atp"], space="PSUM"))

        n_iters = ng * repeat
        st = {}  # per-iteration state

        def emit_qk(i):
            xt_sb = xt_tiles[i]
            s = st[i] = {}
            q_sb, k_sb = [], []
            for which, w_sb, b_sb, dst in (
                ("q", wq_sb, bq_sb, q_sb),
                ("k", wk_sb, bk_sb, k_sb),
            ):
                for h in range(H):
                    ps = ps_big.tile([128, ntok], f32, tag="big", name="qk_ps")
                    for k in range(KE):
                        nc.tensor.matmul(
                            ps,
                            w_sb[k][:, ts(h, 128)],
                            xt_sb[k],
                            start=(k == 0),
                            stop=(k == KE - 1),
                        )
                    t = qkp.tile([128, ntok], f16, tag=f"{which}{h}", name=f"{which}{h}")
                    nc.vector.tensor_scalar_add(t, ps, b_sb[:, h : h + 1])
                    dst.append(t)
            s["q"], s["k"] = q_sb, k_sb

        def emit_v(i):
            s = st[i]
            xt_sb = xt_tiles[i]
            v_sb = []
            for bi in range(gb):
                ps = ps_big.tile([128, E], f32, tag="big", name="v_ps")
                for k in range(KE):
                    nc.tensor.matmul(
                        ps,
                        xt_sb[k][:, ts(bi, 128)],
                        wv_sb[k],
                        start=(k == 0),
                        stop=(k == KE - 1),
                    )
                t = vp.tile([128, E], b16, tag=f"v{bi}", name=f"v{bi}")
                # vector engine (not scalar) so the scalar queue reaches exp()
                # with no backlog — exp is on the attention critical path
                nc.vector.tensor_copy(t, ps)
                v_sb.append(t)
            s["v"] = v_sb
            at_t = atp.tile([128, H, ntok], f16, tag="at", name="at_t")
            s["at_t"] = at_t
            s["at"] = [at_t[:, h, :] for h in range(H)]

        def emit_scores(i, bi):
            # S^T[t, (h,s)] for batch bi, then unnormalized exp in bfloat16
            s = st[i]
            s_ps = ps_sz.tile([128, H * 128], f32, tag="sz", name="s_ps")
            for h in range(H):
                nc.tensor.matmul(
                    s_ps[:, ts(h, 128)],
                    s["k"][h][:, ts(bi, 128)],
                    s["q"][h][:, ts(bi, 128)],
                )
            p_sb = pp.tile([128, H * 128], b16, tag="p", name="p_sb")
            nc.scalar.activation(out=p_sb, in_=s_ps, func=AF.Exp)
            s[f"p{bi}"] = p_sb

        def emit_attn_out(i, bi):
            s = st[i]
            p_sb = s.pop(f"p{bi}")
            # column sums of exp, broadcast to all partitions by the
            # ones-stationary matmul
            z_ps = ps_sz.tile([128, H * 128], f32, tag="sz", name="z_ps")
            nc.tensor.matmul(z_ps, ones, p_sb)
            rz_sb = rzp.tile([128, H * 128], f32, tag="rz", name="rz_sb")
            nc.vector.reciprocal(rz_sb, z_ps)
            at_ps = ps_atp.tile([128, H, 128], f32, tag="atp", name="at_ps")
            for h in range(H):
                nc.tensor.matmul(
                    at_ps[:, h, :], s["v"][bi][:, ts(h, 128)], p_sb[:, ts(h, 128)]
                )
            # normalize while copying out of PSUM: at = at_ps * (1/z)
            nc.vector.tensor_mul(
                s["at_t"][:, :, ts(bi, 128)],
                at_ps,
                rz_sb.rearrange("p (h s) -> p h s", h=H),
            )

        def emit_outproj(i):
            s = st[i]
            xt_sb = xt_tiles.pop(i)
            xm_sb = []
            for m in range(KE):
                ps = ps_big.tile([128, ntok], f32, tag="big", name="o_ps")
                for k in range(KE):
                    nc.tensor.matmul(
                        ps,
                        wo_sb[k][:, ts(m, 128)],
                        s["at"][k],
                        start=(k == 0),
                        stop=(k == KE - 1),
                    )
                tmp = tmpp.tile([128, ntok], f16, tag="tmp", name="tmp")
                nc.scalar.activation(
                    out=tmp, in_=ps, func=AF.Identity, bias=bo_sb[:, m : m + 1]
                )
                xm = xmp.tile([128, ntok], f16, tag=f"xm{m}", name=f"xm{m}")
                nc.gpsimd.tensor_add(xm, tmp, xt_sb[m])
                xm_sb.append(xm)
            s["xm"] = xm_sb
            xm8 = []
            for j in range(nk8 // 2):
                t8 = xmp.tile([128, 2, ntok], mybir.dt.float8e4, tag=f"xm8_{j}", name=f"xm8_{j}")
                for r in range(2):
                    nc.gpsimd.tensor_copy(t8[:, r, :], xm_sb[2 * j + r])
                xm8.append(t8)
            s["xm8"] = xm8

        def emit_mlp1_chunk(i, fs):
            s = st[i]
            h_sb = s.setdefault("h", [])
            for f in fs:
                ps = ps_big.tile([128, ntok], f32, tag="big", name="h_ps")
                for j in range(nk8 // 2):
                    nc.tensor.matmul(
                        ps,
                        w1q_sb[:, 2 * j : 2 * j + 2, ts(f, 128)],
                        s["xm8"][j],
                        start=(j == 0),
                        stop=(nk8 == KE and j == nk8 // 2 - 1),
                        perf_mode=mybir.MatmulPerfMode.DoubleRow,
                    )
                for k in range(nk8, KE):
                    nc.tensor.matmul(
                        ps,
                        w1_sb[k][:, ts(f, 128)],
                        s["xm"][k],
                        start=(k == 0),
                        stop=(k == KE - 1),
                    )
                if f < nf8:
                    # fp8 pair tile feeding MLP2's DoubleRow chain directly
                    j = f // 2
                    if f % 2 == 0:
                        t8 = hp.tile(
                            [128, 2, ntok], mybir.dt.float8e4,
                            tag=f"h8_{j}", name=f"h8_{j}",
                        )
                        s.setdefault("h8", []).append(t8)
                    out_ap = s["h8"][j][:, f % 2, :]
                    t = None
                else:
                    t = hp.tile([128, ntok], f16, tag=f"h{f}", name=f"h{f}")
                    out_ap = t
                if f % 2 == 0:
                    nc.scalar.activation(
                        out=out_ap, in_=ps, func=AF.Relu, bias=b1_sb[:, f : f + 1]
                    )
                else:
                    nc.vector.tensor_scalar(
                        out_ap, ps, b1_sb[:, f : f + 1], 0.0,
                        op0=ALU.add, op1=ALU.max,
                    )
                h_sb.append(t)

        def emit_mlp2_store(i):
            s = st[i]
            g = i % ng
            c0 = g * ntok
            h_sb = s["h"]
            yf = yp.tile([128, KE, ntok], f16, tag="yf", name="yf")
            for m in range(KE):
                ps = ps_big.tile([128, ntok], f32, tag="big", name="acc_ps")
                for j in range(nf8 // 2):
                    nc.tensor.matmul(
                        ps,
                        w2q_sb[:, 2 * j : 2 * j + 2, ts(m, 128)],
                        s["h8"][j],
                        start=(j == 0),
                        stop=(nf8 == KF and j == nf8 // 2 - 1),
                        perf_mode=mybir.MatmulPerfMode.DoubleRow,
                    )
                for f in range(nf8, KF):
                    nc.tensor.matmul(
                        ps,
                        w2_sb[f][:, ts(m, 128)],
                        h_sb[f],
                        start=(f == 0),
                        stop=(f == KF - 1),
                    )
                tmpf = tmpp.tile([128, ntok], f32, tag="tmpf", name="tmpf")
                nc.scalar.activation(
                    out=tmpf, in_=ps, func=AF.Identity, scale=1.0 / 256.0,
                    bias=b2_sb[:, m : m + 1],
                )
                nc.gpsimd.tensor_add(yf[:, m, :], tmpf, s["xm"][m])
                # store each 128-row block as soon as its residual add lands
                nc.scalar.dma_start(
                    out=yt[m * 128 : (m + 1) * 128, c0 : c0 + ntok],
                    in_=yf[:, m, :],
                )
            del st[i]

        for i in range(n_iters):
            if i + 1 < n_iters:
                emit_load(i + 1)
            emit_qk(i)
            emit_v(i)
            for bi in range(gb):
                emit_scores(i, bi)
            for bi in range(gb):
                emit_attn_out(i, bi)
            if i >= 1:
                emit_mlp1_chunk(i - 1, range(KF))
            emit_outproj(i)
            if i >= 1:
                emit_mlp2_store(i - 1)
        emit_mlp1_chunk(n_iters - 1, range(KF))
        emit_mlp2_store(n_iters - 1)

    if mwfix:
        _fix_multiwaits(nc)
    return nc


def _get_program(ng, variant="full", repeat=1, cfg=None):
    key = ("nc", ng, variant, repeat, tuple(sorted((cfg or {}).items())))
    if key not in _cache:
        _cache[key] = _build(ng, variant, repeat, cfg)
    return _cache[key]


def make_in_maps(x, wq, bq, wk, bk, wv, bv, wo, bo, w1, b1, w2, b2):
    x = np.asarray(x, np.float32)
    to_bf = lambda a: np.ascontiguousarray(np.asarray(a, np.float32).astype(MMDT_NP))

    # host-side prep: shard + transpose + cast
    ntok_total = BL * S
    x_sh = x.reshape(NCORES, ntok_total, E)
    xts = [np.ascontiguousarray(x_sh[c].T).astype(MMDT_NP) for c in range(NCORES)]

    # MLP scale convention: w1 (and b1) carry x16 so the fp8 copy of w1 sits
    # in e4m3's normal range; h tiles then hold 16*h and w2 carries /16.
    # Both scalings are exact powers of two in fp16.
    import ml_dtypes

    w1_s = np.asarray(w1, np.float32) * 16.0
    w2_s = np.asarray(w2, np.float32) * 16.0
    wq_b, wk_b, wv_b, wo_b, w1_b, w2_b = map(
        to_bf, (wq, wk, wv, wo, w1_s, w2_s)
    )
    w1q = np.ascontiguousarray(
        w1_s.reshape(KE, 128, F).transpose(1, 0, 2)
    ).astype(ml_dtypes.float8_e4m3)
    w2q = np.ascontiguousarray(
        w2_s.reshape(KF, 128, E).transpose(1, 0, 2)
    ).astype(ml_dtypes.float8_e4m3)

    resh = lambda b, nk: np.asarray(b, np.float32).reshape(nk, 128).T
    # bv is folded into the output-projection bias: P rows sum to 1, so
    # attn@wo + bo == (P@v_nobias)@wo + (bv@wo + bo).
    bo_eff = (
        np.asarray(bv, np.float64) @ np.asarray(wo, np.float64)
        + np.asarray(bo, np.float64)
    ).astype(np.float32)
    bias_pack = np.zeros((128, 32), np.float32)
    bias_pack[:, 0:KE] = resh(bq, KE)
    bias_pack[:, KE : 2 * KE] = resh(bk, KE)
    bias_pack[:, 2 * KE : 3 * KE] = resh(bo_eff, KE)
    bias_pack[:, 12 : 12 + KF] = resh(np.asarray(b1, np.float32) * 16.0, KF)
    bias_pack[:, 28 : 28 + KE] = resh(b2, KE)

    in_maps = []
    for c in range(NCORES):
        in_maps.append(
            {
                "xt": xts[c],
                "wq": wq_b,
                "wk": wk_b,
                "wv": wv_b,
                "wo": wo_b,
                "w1": w1_b,
                "w1q": w1q,
                "w2": w2_b,
                "w2q": w2q,
                "bias": bias_pack,
            }
        )
    return in_maps


def kernel(
    x, wq, bq, wk, bk, wv, bv, wo, bo, w1, b1, w2, b2, _ng=BL // GB
):
    import os

    from concourse.bass_utils import run_bass_kernel_spmd

    # The NTFF trace hook module does not exist in this container; make sure
    # run_bass_kernel_spmd never takes the trace branch even if BASS_TRACE
    # is set in the environment.
    os.environ["BASS_NEVER_TRACE"] = "1"

    in_maps = make_in_maps(x, wq, bq, wk, bk, wv, bv, wo, bo, w1, b1, w2, b2)
    ntok_total = BL * S
    nc = _get_program(_ng)

    res = run_bass_kernel_spmd(nc, in_maps, core_ids=list(range(NCORES)))
    _cache["last_result"] = res

    out = np.empty((NCORES, ntok_total, E), np.float32)
    for c in range(NCORES):
        out[c] = res.results[c]["yt"].T.astype(np.float32)
    return out.reshape(B, S, E)
